# revision 1
# baseline (speedup 1.0000x reference)
import numpy as np
import jax
import jax.numpy as jnp
from functools import partial

B, C, X, Y = 4, 16, 256, 256
TWO_PI = 2.0 * np.pi
NDEV = 8
XH = X // 2  # x-half per device


def _w_matrix():
    # img[x'] = ifftshift(ifft(ifftshift(v)))[x'] = sum_j W[x',j] v[j]
    # W[x',j] = (1/N) exp(+2pi i ((j+N/2)%N)((x'+N/2)%N)/N)
    n = X
    j = (np.arange(n) + n // 2) % n
    xp = (np.arange(n) + n // 2) % n
    ang = TWO_PI * np.outer(xp, j) / n
    Wr = (np.cos(ang) / n).astype(np.float32)
    Wi = (np.sin(ang) / n).astype(np.float32)
    return Wr, Wi


@partial(jax.pmap, axis_name='d')
def _shard_fn(kdr, kdi, csr, csi, mk, fl, fmr, fmi, te, Wr, Wi):
    # kdr/kdi: [C,X,Y] full-X kdata for this b
    # csr/csi: [C,XH,Y]; mk/fl: [XH,Y]; fmr/fmi: [Y,Y]; te: [Y]
    # Wr/Wi: [XH,X] per-device IDFT rows (with ifftshifts folded in)
    # IDFT along X -> img[c,x',t]
    imgr = jnp.einsum('px,cxy->cpy', Wr, kdr) - jnp.einsum('px,cxy->cpy', Wi, kdi)
    imgi = jnp.einsum('px,cxy->cpy', Wr, kdi) + jnp.einsum('px,cxy->cpy', Wi, kdr)
    # -> [x', t, c]
    Kr = imgr.transpose(1, 2, 0) * mk[:, :, None]
    Ki = imgi.transpose(1, 2, 0) * mk[:, :, None]
    # phase factor exp(i*2pi*te[t]*field[x',y]) -> [x', t, y]
    ph = TWO_PI * te[None, :, None] * fl[:, None, :]
    cph = jnp.cos(ph)
    sph = jnp.sin(ph)
    # A = fmt[t,y] * fm[x',t,y] * mask[x',t]
    Ar = (fmr[None] * cph - fmi[None] * sph) * mk[:, :, None]
    Ai = (fmr[None] * sph + fmi[None] * cph) * mk[:, :, None]
    # ci[x',y,c] = sum_t conj(A)[x',t,y] * K[x',t,c]
    cir = jnp.einsum('pty,ptc->pyc', Ar, Kr) + jnp.einsum('pty,ptc->pyc', Ai, Ki)
    cii = jnp.einsum('pty,ptc->pyc', Ar, Ki) - jnp.einsum('pty,ptc->pyc', Ai, Kr)
    # rec[x',y] = sum_c ci * conj(csm_t); csm_t[x',y,c] = csm[c,x',y]
    ctr = csr.transpose(1, 2, 0)
    cti = csi.transpose(1, 2, 0)
    recr = jnp.sum(cir * ctr + cii * cti, axis=-1)
    reci = jnp.sum(cii * ctr - cir * cti, axis=-1)
    return recr, reci


def kernel(kdata_r, kdata_i, csm_r, csm_i, mask, field, fmt_r, fmt_i, tl,
           bool_updown):
    kdata_r = np.asarray(kdata_r, np.float32)
    kdata_i = np.asarray(kdata_i, np.float32)
    csm_r = np.asarray(csm_r, np.float32)
    csm_i = np.asarray(csm_i, np.float32)
    mask = np.asarray(mask, np.float32)
    field = np.asarray(field, np.float32)
    fmt_r = np.asarray(fmt_r, np.float32)
    fmt_i = np.asarray(fmt_i, np.float32)
    tl = np.asarray(tl, np.float32)
    te = tl if bool(bool_updown) else tl[::-1].copy()

    Wr, Wi = _w_matrix()
    # device d -> (b = d//2, xh = d%2)
    st = lambda parts: np.stack(parts, 0)
    kdr_s = st([kdata_r[d // 2] for d in range(NDEV)])
    kdi_s = st([kdata_i[d // 2] for d in range(NDEV)])
    sl = lambda a, d: a[d // 2, :, (d % 2) * XH:(d % 2 + 1) * XH, :]
    csr_s = st([sl(csm_r, d) for d in range(NDEV)])
    csi_s = st([sl(csm_i, d) for d in range(NDEV)])
    mk_s = st([mask[d // 2, (d % 2) * XH:(d % 2 + 1) * XH, :] for d in range(NDEV)])
    fl_s = st([field[d // 2, (d % 2) * XH:(d % 2 + 1) * XH, :] for d in range(NDEV)])
    fmr_s = st([fmt_r] * NDEV)
    fmi_s = st([fmt_i] * NDEV)
    te_s = st([te] * NDEV)
    Wr_s = st([Wr[(d % 2) * XH:(d % 2 + 1) * XH, :] for d in range(NDEV)])
    Wi_s = st([Wi[(d % 2) * XH:(d % 2 + 1) * XH, :] for d in range(NDEV)])

    recr, reci = _shard_fn(kdr_s, kdi_s, csr_s, csi_s, mk_s, fl_s, fmr_s,
                           fmi_s, te_s, Wr_s, Wi_s)
    recr = np.asarray(recr)
    reci = np.asarray(reci)
    out = np.empty((B, X, Y), np.complex64)
    for d in range(NDEV):
        b, xh = d // 2, d % 2
        out[b, xh * XH:(xh + 1) * XH, :] = recr[d] + 1j * reci[d]
    return out
